# revision 32
# baseline (speedup 1.0000x reference)
"""Trainium2 Bass kernel for AttentiveTransformer:
   out = sparsemax(GBN(feat @ W.T) * priors)

Data-parallel over 8 NeuronCores: batch 131072 rows -> 8 shards of 16384.
Per core: 32 superchunks x 512 rows; each superchunk = 4 GBN chunks (VBS=128).

v4 = baseline dataflow with measured-positive upgrades:
  - float32r GEMM (1 cyc/row vs 4 for fp32; halves the LDWEIGHTS traffic)
  - x PSUM banks freed immediately by the ACT copy pass; bn_stats and the
    ACT normalize both read the SBUF copy (PSUM reads on ACT/DVE are
    ~200ns slower on HW), so the stats->merge->normalize latency is off
    the inter-superchunk critical cycle
  - batched tau tail on GpSimd (plain add/sub/mult TTs only)
  - z-mult split: 2 chunks DVE tensor_tensor, 2 chunks ACT-copy + GpSimd
    in-place multiply, balancing DVE/ACT/GP occupancy
"""
import sys

sys.path.insert(0, "/opt/trn_rl_repo")

import numpy as np
from contextlib import ExitStack

import concourse.bass as bass
import concourse.bacc as bacc
import concourse.tile as tile
from concourse.tile import add_dep_helper
from concourse import mybir
from concourse.bass_utils import run_bass_kernel_spmd

f32 = mybir.dt.float32
fr32 = mybir.dt.float32r
AF = mybir.ActivationFunctionType
OP = mybir.AluOpType

N_CORES = 8
B, IN, D = 131072, 128, 512
ROWS = B // N_CORES          # 16384 rows per core
SC_ROWS = 512                # superchunk rows (4 GBN chunks)
N_SC = ROWS // SC_ROWS       # 32
VBS = 128
EPS = 1e-5
NEG = -1.0e9

ZMULT_ON_GP = ()         # chunks whose z-mult runs ACT-copy + GpSimd
RELU_ON_DVE = (1, 3)         # chunks whose final relu runs on DVE tensor_scalar


def build_nc():
    nc = bacc.Bacc(None, target_bir_lowering=False)

    priors = nc.dram_tensor("priors", [ROWS, D], f32, kind="ExternalInput")
    feat = nc.dram_tensor("processed_feat", [ROWS, IN], f32, kind="ExternalInput")
    Wd = nc.dram_tensor("W", [D, IN], f32, kind="ExternalInput")
    gam = nc.dram_tensor("gamma", [D], f32, kind="ExternalInput")
    bet = nc.dram_tensor("beta", [D], f32, kind="ExternalInput")
    out = nc.dram_tensor("out", [ROWS, D], f32, kind="ExternalOutput")

    with tile.TileContext(nc) as tc, ExitStack() as ctx:
        singles = ctx.enter_context(tc.tile_pool(name="singles", bufs=1))
        f4_pool = ctx.enter_context(tc.tile_pool(name="f4", bufs=4))
        ft_pool = ctx.enter_context(tc.tile_pool(name="ft", bufs=3))
        xs_pool = ctx.enter_context(tc.tile_pool(name="xs", bufs=5))
        xn_pool = ctx.enter_context(tc.tile_pool(name="xn", bufs=8))
        p_pool = ctx.enter_context(tc.tile_pool(name="p", bufs=4))
        z_pool = ctx.enter_context(tc.tile_pool(name="z", bufs=8))
        o_pool = ctx.enter_context(tc.tile_pool(name="o", bufs=4))
        st_pool = ctx.enter_context(tc.tile_pool(name="st", bufs=3))
        sm_pool = ctx.enter_context(tc.tile_pool(name="sm", bufs=16))
        ps_scr = ctx.enter_context(tc.tile_pool(name="psscr", bufs=2, space="PSUM"))
        ps_x = ctx.enter_context(tc.tile_pool(name="psx", bufs=2, space="PSUM"))
        ps_zt = ctx.enter_context(tc.tile_pool(name="pszt", bufs=2, space="PSUM"))

        # ---------- one-time constants ----------
        ident = singles.tile([128, 128], f32)
        nc.gpsimd.iota(ident, [[1, 128]], base=0, channel_multiplier=-1,
                       allow_small_or_imprecise_dtypes=True)
        nc.vector.tensor_scalar(ident, ident, 0.0, None, OP.is_equal)

        # WT [128k, 512d] resident, fp32r for the 1 cyc/row GEMM
        WT = singles.tile([128, D], fr32)
        wtp = ps_scr.tile([128, D], f32, tag="scr")
        for s in range(4):
            wtile = ft_pool.tile([128, 128], f32, tag="wtile")
            nc.sync.dma_start(out=wtile, in_=Wd[s * 128:(s + 1) * 128, :])
            nc.tensor.transpose(wtp[:, s * 128:(s + 1) * 128], wtile, ident)
        nc.scalar.copy(WT, wtp)

        # gamma/beta broadcast [128, 4slice, 4chunk]
        gamma44 = singles.tile([128, 4, 4], f32)
        beta44 = singles.tile([128, 4, 4], f32)
        gamma4 = singles.tile([128, 4], f32)
        beta4 = singles.tile([128, 4], f32)
        gr = gam.rearrange("(s p) -> s p", p=128)
        br = bet.rearrange("(s p) -> s p", p=128)
        for s4 in range(4):
            nc.sync.dma_start(out=gamma4[:, s4:s4 + 1],
                              in_=gr[s4].rearrange("(p o) -> p o", o=1))
            nc.sync.dma_start(out=beta4[:, s4:s4 + 1],
                              in_=br[s4].rearrange("(p o) -> p o", o=1))
        for c4 in range(4):
            nc.vector.tensor_copy(gamma44[:, :, c4], gamma4)
            nc.vector.tensor_copy(beta44[:, :, c4], beta4)

        eps_t = singles.tile([128, 1], f32)
        nc.vector.memset(eps_t, EPS)
        c32 = singles.tile([128, 4, 4], f32)
        nc.vector.memset(c32, 32.0)
        cnh = singles.tile([128, 4, 4], f32)
        nc.vector.memset(cnh, -0.5)

        # rho / 1/rho replicated over the 4 chunks: [128, 4, 16]
        rho16 = singles.tile([128, 16], f32)
        nc.gpsimd.iota(rho16, [[1, 16]], base=1, channel_multiplier=0,
                       allow_small_or_imprecise_dtypes=True)
        invrho = singles.tile([128, 16], f32)
        nc.vector.reciprocal(invrho, rho16)
        rho16p = singles.tile([128, 4, 16], f32)
        invrhop = singles.tile([128, 4, 16], f32)
        for jj in range(4):
            nc.vector.tensor_copy(rho16p[:, jj], rho16)
            nc.vector.tensor_copy(invrhop[:, jj], invrho)

        fe_r = feat.rearrange("(n c p) k -> n p c k", p=128, c=4)
        pr_r = priors.rearrange("(n c p) d -> n p c d", p=128, c=4)
        out_r = out.rearrange("(n c p) d -> n p c d", p=128, c=4)

        def emit_load(sc):
            f4 = f4_pool.tile([128, 4, 128], f32, tag="f4")
            nc.sync.dma_start(out=f4, in_=fe_r[sc])
            ftp = ps_scr.tile([128, SC_ROWS], f32, tag="scr")
            for q in range(4):
                nc.tensor.transpose(ftp[:, q * 128:(q + 1) * 128],
                                    f4[:, q], ident)
            featT = ft_pool.tile([128, SC_ROWS], fr32, tag="featT")
            nc.scalar.copy(featT, ftp)
            p4 = p_pool.tile([128, 4, D], f32)
            nc.sync.dma_start(out=p4, in_=pr_r[sc])
            stats = st_pool.tile([128, 4, 4, 6], f32)
            return dict(featT=featT, p4=p4, stats=stats, xss=[], bn_insts=[])

        def emit_gemm_stats(st, sp):
            # slice pair sp covers slices 2*sp, 2*sp+1 through a 2-bank
            # PSUM tile drained by a single [128,1024] ACT copy
            xp2 = ps_x.tile([128, 2, SC_ROWS], f32)
            for j in range(2):
                s = 2 * sp + j
                nc.tensor.matmul(xp2[:, j], WT[:, s * 128:(s + 1) * 128],
                                 st["featT"])
            xs2 = xs_pool.tile([128, 2, SC_ROWS], f32)
            nc.scalar.copy(xs2, xp2)
            st["xss"].extend([xs2[:, 0], xs2[:, 1]])
            for j in range(2):
                s = 2 * sp + j
                for c in range(4):
                    bi = nc.vector.bn_stats(
                        out=st["stats"][:, s, c],
                        in_=xs2[:, j, c * VBS:(c + 1) * VBS])
                    st["bn_insts"].append(bi)

        def emit_stats_merge(st):
            stats = st["stats"]
            me = stats[:, :, :, 1]
            mo = stats[:, :, :, 4]
            M2e = stats[:, :, :, 2]
            M2o = stats[:, :, :, 5]
            dm = sm_pool.tile([128, 4, 4], f32, tag="dm")
            m2 = sm_pool.tile([128, 4, 4], f32, tag="m2")
            sm = sm_pool.tile([128, 4, 4], f32, tag="sm")
            sd = sm_pool.tile([128, 4, 4], f32, tag="sd")
            isd = sm_pool.tile([128, 4, 4], f32, tag="isd")
            sscale = sm_pool.tile([128, 4, 4], f32, tag="sscale")
            tshift = sm_pool.tile([128, 4, 4], f32, tag="tshift")
            i1 = nc.gpsimd.tensor_tensor(dm, me, mo, OP.subtract)
            i2 = nc.gpsimd.tensor_tensor(m2, M2e, M2o, OP.add)
            i3 = nc.gpsimd.tensor_tensor(sm, me, mo, OP.add)
            for bi in st["bn_insts"]:
                add_dep_helper(i1.ins, bi.ins, sync=True, reason="stats raw")
                add_dep_helper(i2.ins, bi.ins, sync=True, reason="stats raw")
                add_dep_helper(i3.ins, bi.ins, sync=True, reason="stats raw")
            nc.gpsimd.tensor_tensor(dm, dm, dm, OP.mult)
            nc.gpsimd.tensor_tensor(dm, dm, c32, OP.mult)
            nc.gpsimd.tensor_tensor(m2, dm, m2, OP.add)
            nc.scalar.activation(sd, m2, AF.Sqrt, bias=eps_t, scale=1.0 / VBS)
            nc.vector.reciprocal(isd, sd)
            nc.gpsimd.tensor_tensor(sscale, isd, gamma44, OP.mult)
            nc.gpsimd.tensor_tensor(sm, sm, sscale, OP.mult)
            nc.gpsimd.tensor_tensor(sm, sm, cnh, OP.mult)
            nc.gpsimd.tensor_tensor(tshift, beta44, sm, OP.add)
            st["sscale"] = sscale
            st["tshift"] = tshift

        def emit_chunk(st, c):
            sscale = st["sscale"]
            tshift = st["tshift"]
            xnc = xn_pool.tile([128, 4, VBS], f32, tag="xnc")
            for s in range(4):
                nc.scalar.activation(
                    out=xnc[:, s],
                    in_=st["xss"][s][:, c * VBS:(c + 1) * VBS],
                    func=AF.Identity, bias=tshift[:, s, c:c + 1],
                    scale=sscale[:, s, c:c + 1])
            ztp = ps_zt.tile([128, D], f32)
            for s in range(4):
                nc.tensor.transpose(ztp[:, s * 128:(s + 1) * 128],
                                    xnc[:, s], ident)
            z = z_pool.tile([128, D], f32, tag="z")
            if c in ZMULT_ON_GP:
                # GpSimd cannot read PSUM: ACT copies, GP multiplies in place
                nc.scalar.copy(z, ztp)
                nc.gpsimd.tensor_tensor(z, z, st["p4"][:, c], OP.mult)
            else:
                nc.vector.tensor_tensor(z, ztp, st["p4"][:, c], OP.mult)
            st["z"][c] = z

            # top-16 extraction into t16[:, c, :]
            t16 = st["t16"]
            cs = st["cs"]
            l1 = sm_pool.tile([128, 32], f32, tag="l1")
            for s in range(4):
                nc.vector.max(out=l1[:, s * 8:(s + 1) * 8],
                              in_=z[:, s * 128:(s + 1) * 128])
            nc.vector.max(out=t16[:, c, 0:8], in_=l1)
            sup = sm_pool.tile([128, 32], f32, tag="sup")
            nc.vector.match_replace(sup, t16[:, c, 0:8], l1, NEG)
            nc.vector.max(out=t16[:, c, 8:16], in_=sup)
            nc.vector.tensor_tensor_scan(cs[:, c], t16[:, c], t16[:, c],
                                         -1.0, OP.add, OP.bypass)

        def emit_tail(st, sci):
            # batched tau computation over [128, 4, 16]
            t16 = st["t16"]
            cs = st["cs"]
            rz = sm_pool.tile([128, 4, 16], f32, tag="rz")
            nc.gpsimd.tensor_tensor(rz, t16, rho16p, OP.mult)
            cond = sm_pool.tile([128, 4, 17], f32, tag="cond")
            nc.gpsimd.memset(cond[:, :, 16:17], 0.0)
            nc.vector.tensor_tensor(cond[:, :, 0:16], rz, cs, OP.is_gt)
            dcn = sm_pool.tile([128, 4, 16], f32, tag="dcn")
            nc.gpsimd.tensor_tensor(dcn, cond[:, :, 1:17], cond[:, :, 0:16],
                                    OP.subtract)
            tauj = sm_pool.tile([128, 4, 16], f32, tag="tauj")
            nc.gpsimd.tensor_tensor(tauj, cs, invrhop, OP.mult)
            scr = sm_pool.tile([128, 4, 16], f32, tag="scr")
            nc.gpsimd.tensor_tensor(scr, tauj, dcn, OP.mult)
            negtau = sm_pool.tile([128, 4], f32, tag="negtau")
            nc.vector.tensor_reduce(out=negtau, in_=scr,
                                    axis=mybir.AxisListType.X, op=OP.add)
            o4 = o_pool.tile([128, 4, D], f32, name="o4", tag="o4")
            for c in range(4):
                if c in RELU_ON_DVE:
                    nc.vector.tensor_scalar(o4[:, c], st["z"][c],
                                            negtau[:, c:c + 1], 0.0,
                                            OP.add, OP.max)
                else:
                    nc.scalar.activation(o4[:, c], st["z"][c], AF.Relu,
                                         bias=negtau[:, c:c + 1], scale=1.0)
            # out-DMA on the GpSimd sw-DGE queue: its wait-for-relu must not
            # head-of-line-block the input DMAs on the Sync queue
            nc.gpsimd.dma_start(out=out_r[sci], in_=o4)

        # ---------- main loop (1-sc software pipeline skew) ----------
        def start_pend(st):
            st["z"] = {}
            st["t16"] = sm_pool.tile([128, 4, 16], f32, name="t16", tag="t16")
            st["cs"] = sm_pool.tile([128, 4, 16], f32, name="cs", tag="cs")

        pend = None
        for sc in range(N_SC):
            st = emit_load(sc)
            if pend is not None:
                start_pend(pend)
            for i in range(4):
                if pend is not None:
                    emit_chunk(pend, i)
                if i < 2:
                    emit_gemm_stats(st, i)
            emit_stats_merge(st)
            if pend is not None:
                emit_tail(pend, sc - 1)
            pend = st
        start_pend(pend)
        for i in range(4):
            emit_chunk(pend, i)
        emit_tail(pend, N_SC - 1)

    nc.compile()
    return nc


_NC_CACHE = None


def kernel(**inputs) -> np.ndarray:
    global _NC_CACHE
    if _NC_CACHE is None:
        _NC_CACHE = build_nc()
    nc = _NC_CACHE

    priors = np.ascontiguousarray(inputs["priors"], dtype=np.float32)
    feat = np.ascontiguousarray(inputs["processed_feat"], dtype=np.float32)
    W = np.ascontiguousarray(inputs["W"], dtype=np.float32)
    gamma = np.ascontiguousarray(inputs["gamma"], dtype=np.float32)
    beta = np.ascontiguousarray(inputs["beta"], dtype=np.float32)

    in_maps = []
    for i in range(N_CORES):
        sl = slice(i * ROWS, (i + 1) * ROWS)
        in_maps.append({
            "priors": priors[sl],
            "processed_feat": feat[sl],
            "W": W,
            "gamma": gamma,
            "beta": beta,
        })
    res = run_bass_kernel_spmd(nc, in_maps, core_ids=list(range(N_CORES)))
    return np.concatenate([r["out"] for r in res.results], axis=0)


if __name__ == "__main__":
    rng = np.random.default_rng(0)
    inputs = {
        "priors": rng.random((B, D), dtype=np.float32),
        "processed_feat": rng.standard_normal((B, IN), dtype=np.float32),
        "W": (rng.standard_normal((D, IN), dtype=np.float32) * 0.1),
        "gamma": np.ones(D, dtype=np.float32),
        "beta": np.zeros(D, dtype=np.float32),
    }
    out = kernel(**inputs)
    print("out", out.shape, out.dtype, float(out.sum()))


# revision 33
# speedup vs baseline: 1.0141x; 1.0141x over previous
"""Trainium2 Bass kernel for AttentiveTransformer:
   out = sparsemax(GBN(feat @ W.T) * priors)

Data-parallel over 8 NeuronCores: batch 131072 rows -> 8 shards of 16384.
Per core: 32 superchunks x 512 rows; each superchunk = 4 GBN chunks (VBS=128).

v4 = baseline dataflow with measured-positive upgrades:
  - float32r GEMM (1 cyc/row vs 4 for fp32; halves the LDWEIGHTS traffic)
  - x PSUM banks freed immediately by the ACT copy pass; bn_stats and the
    ACT normalize both read the SBUF copy (PSUM reads on ACT/DVE are
    ~200ns slower on HW), so the stats->merge->normalize latency is off
    the inter-superchunk critical cycle
  - batched tau tail on GpSimd (plain add/sub/mult TTs only)
  - z-mult split: 2 chunks DVE tensor_tensor, 2 chunks ACT-copy + GpSimd
    in-place multiply, balancing DVE/ACT/GP occupancy
"""
import sys

sys.path.insert(0, "/opt/trn_rl_repo")

import numpy as np
from contextlib import ExitStack

import concourse.bass as bass
import concourse.bacc as bacc
import concourse.tile as tile
from concourse.tile import add_dep_helper
from concourse import mybir
from concourse.bass_utils import run_bass_kernel_spmd

f32 = mybir.dt.float32
fr32 = mybir.dt.float32r
AF = mybir.ActivationFunctionType
OP = mybir.AluOpType

N_CORES = 8
B, IN, D = 131072, 128, 512
ROWS = B // N_CORES          # 16384 rows per core
SC_ROWS = 512                # superchunk rows (4 GBN chunks)
N_SC = ROWS // SC_ROWS       # 32
VBS = 128
EPS = 1e-5
NEG = -1.0e9

ZMULT_ON_GP = (1, 3)         # chunks whose z-mult runs ACT-copy + GpSimd
RELU_ON_DVE = (1, 3)         # chunks whose final relu runs on DVE tensor_scalar


def build_nc():
    nc = bacc.Bacc(None, target_bir_lowering=False)

    priors = nc.dram_tensor("priors", [ROWS, D], f32, kind="ExternalInput")
    feat = nc.dram_tensor("processed_feat", [ROWS, IN], f32, kind="ExternalInput")
    Wd = nc.dram_tensor("W", [D, IN], f32, kind="ExternalInput")
    gam = nc.dram_tensor("gamma", [D], f32, kind="ExternalInput")
    bet = nc.dram_tensor("beta", [D], f32, kind="ExternalInput")
    out = nc.dram_tensor("out", [ROWS, D], f32, kind="ExternalOutput")

    with tile.TileContext(nc) as tc, ExitStack() as ctx:
        singles = ctx.enter_context(tc.tile_pool(name="singles", bufs=1))
        f4_pool = ctx.enter_context(tc.tile_pool(name="f4", bufs=4))
        ft_pool = ctx.enter_context(tc.tile_pool(name="ft", bufs=3))
        xs_pool = ctx.enter_context(tc.tile_pool(name="xs", bufs=5))
        xn_pool = ctx.enter_context(tc.tile_pool(name="xn", bufs=8))
        p_pool = ctx.enter_context(tc.tile_pool(name="p", bufs=4))
        z_pool = ctx.enter_context(tc.tile_pool(name="z", bufs=8))
        o_pool = ctx.enter_context(tc.tile_pool(name="o", bufs=4))
        st_pool = ctx.enter_context(tc.tile_pool(name="st", bufs=3))
        sm_pool = ctx.enter_context(tc.tile_pool(name="sm", bufs=16))
        ps_scr = ctx.enter_context(tc.tile_pool(name="psscr", bufs=2, space="PSUM"))
        ps_x = ctx.enter_context(tc.tile_pool(name="psx", bufs=2, space="PSUM"))
        ps_zt = ctx.enter_context(tc.tile_pool(name="pszt", bufs=2, space="PSUM"))

        # ---------- one-time constants ----------
        ident = singles.tile([128, 128], f32)
        nc.gpsimd.iota(ident, [[1, 128]], base=0, channel_multiplier=-1,
                       allow_small_or_imprecise_dtypes=True)
        nc.vector.tensor_scalar(ident, ident, 0.0, None, OP.is_equal)

        # WT [128k, 512d] resident, fp32r for the 1 cyc/row GEMM
        WT = singles.tile([128, D], fr32)
        wtp = ps_scr.tile([128, D], f32, tag="scr")
        for s in range(4):
            wtile = ft_pool.tile([128, 128], f32, tag="wtile")
            nc.sync.dma_start(out=wtile, in_=Wd[s * 128:(s + 1) * 128, :])
            nc.tensor.transpose(wtp[:, s * 128:(s + 1) * 128], wtile, ident)
        nc.scalar.copy(WT, wtp)

        # gamma/beta broadcast [128, 4slice, 4chunk]
        gamma44 = singles.tile([128, 4, 4], f32)
        beta44 = singles.tile([128, 4, 4], f32)
        gamma4 = singles.tile([128, 4], f32)
        beta4 = singles.tile([128, 4], f32)
        gr = gam.rearrange("(s p) -> s p", p=128)
        br = bet.rearrange("(s p) -> s p", p=128)
        for s4 in range(4):
            nc.sync.dma_start(out=gamma4[:, s4:s4 + 1],
                              in_=gr[s4].rearrange("(p o) -> p o", o=1))
            nc.sync.dma_start(out=beta4[:, s4:s4 + 1],
                              in_=br[s4].rearrange("(p o) -> p o", o=1))
        for c4 in range(4):
            nc.vector.tensor_copy(gamma44[:, :, c4], gamma4)
            nc.vector.tensor_copy(beta44[:, :, c4], beta4)

        eps_t = singles.tile([128, 1], f32)
        nc.vector.memset(eps_t, EPS)
        c32 = singles.tile([128, 4, 4], f32)
        nc.vector.memset(c32, 32.0)
        cnh = singles.tile([128, 4, 4], f32)
        nc.vector.memset(cnh, -0.5)

        # rho / 1/rho replicated over the 4 chunks: [128, 4, 16]
        rho16 = singles.tile([128, 16], f32)
        nc.gpsimd.iota(rho16, [[1, 16]], base=1, channel_multiplier=0,
                       allow_small_or_imprecise_dtypes=True)
        invrho = singles.tile([128, 16], f32)
        nc.vector.reciprocal(invrho, rho16)
        rho16p = singles.tile([128, 4, 16], f32)
        invrhop = singles.tile([128, 4, 16], f32)
        for jj in range(4):
            nc.vector.tensor_copy(rho16p[:, jj], rho16)
            nc.vector.tensor_copy(invrhop[:, jj], invrho)

        fe_r = feat.rearrange("(n c p) k -> n p c k", p=128, c=4)
        pr_r = priors.rearrange("(n c p) d -> n p c d", p=128, c=4)
        out_r = out.rearrange("(n c p) d -> n p c d", p=128, c=4)

        def emit_load(sc):
            f4 = f4_pool.tile([128, 4, 128], f32, tag="f4")
            nc.sync.dma_start(out=f4, in_=fe_r[sc])
            ftp = ps_scr.tile([128, SC_ROWS], f32, tag="scr")
            for q in range(4):
                nc.tensor.transpose(ftp[:, q * 128:(q + 1) * 128],
                                    f4[:, q], ident)
            featT = ft_pool.tile([128, SC_ROWS], fr32, tag="featT")
            nc.scalar.copy(featT, ftp)
            p4 = p_pool.tile([128, 4, D], f32)
            nc.sync.dma_start(out=p4, in_=pr_r[sc])
            stats = st_pool.tile([128, 4, 4, 6], f32)
            return dict(featT=featT, p4=p4, stats=stats, xss=[], bn_insts=[])

        def emit_gemm_stats(st, sp):
            # slice pair sp covers slices 2*sp, 2*sp+1 through a 2-bank
            # PSUM tile drained by a single [128,1024] ACT copy
            xp2 = ps_x.tile([128, 2, SC_ROWS], f32)
            for j in range(2):
                s = 2 * sp + j
                nc.tensor.matmul(xp2[:, j], WT[:, s * 128:(s + 1) * 128],
                                 st["featT"])
            xs2 = xs_pool.tile([128, 2, SC_ROWS], f32)
            nc.scalar.copy(xs2, xp2)
            st["xss"].extend([xs2[:, 0], xs2[:, 1]])
            for j in range(2):
                s = 2 * sp + j
                for c in range(4):
                    bi = nc.vector.bn_stats(
                        out=st["stats"][:, s, c],
                        in_=xs2[:, j, c * VBS:(c + 1) * VBS])
                    st["bn_insts"].append(bi)

        def emit_stats_merge(st):
            stats = st["stats"]
            me = stats[:, :, :, 1]
            mo = stats[:, :, :, 4]
            M2e = stats[:, :, :, 2]
            M2o = stats[:, :, :, 5]
            dm = sm_pool.tile([128, 4, 4], f32, tag="dm")
            m2 = sm_pool.tile([128, 4, 4], f32, tag="m2")
            sm = sm_pool.tile([128, 4, 4], f32, tag="sm")
            sd = sm_pool.tile([128, 4, 4], f32, tag="sd")
            isd = sm_pool.tile([128, 4, 4], f32, tag="isd")
            sscale = sm_pool.tile([128, 4, 4], f32, tag="sscale")
            tshift = sm_pool.tile([128, 4, 4], f32, tag="tshift")
            i1 = nc.gpsimd.tensor_tensor(dm, me, mo, OP.subtract)
            i2 = nc.gpsimd.tensor_tensor(m2, M2e, M2o, OP.add)
            i3 = nc.gpsimd.tensor_tensor(sm, me, mo, OP.add)
            for bi in st["bn_insts"]:
                add_dep_helper(i1.ins, bi.ins, sync=True, reason="stats raw")
                add_dep_helper(i2.ins, bi.ins, sync=True, reason="stats raw")
                add_dep_helper(i3.ins, bi.ins, sync=True, reason="stats raw")
            nc.gpsimd.tensor_tensor(dm, dm, dm, OP.mult)
            nc.gpsimd.tensor_tensor(dm, dm, c32, OP.mult)
            nc.gpsimd.tensor_tensor(m2, dm, m2, OP.add)
            nc.scalar.activation(sd, m2, AF.Sqrt, bias=eps_t, scale=1.0 / VBS)
            nc.vector.reciprocal(isd, sd)
            nc.gpsimd.tensor_tensor(sscale, isd, gamma44, OP.mult)
            nc.gpsimd.tensor_tensor(sm, sm, sscale, OP.mult)
            nc.gpsimd.tensor_tensor(sm, sm, cnh, OP.mult)
            nc.gpsimd.tensor_tensor(tshift, beta44, sm, OP.add)
            st["sscale"] = sscale
            st["tshift"] = tshift

        def emit_chunk(st, c):
            sscale = st["sscale"]
            tshift = st["tshift"]
            xnc = xn_pool.tile([128, 4, VBS], f32, tag="xnc")
            for s in range(4):
                nc.scalar.activation(
                    out=xnc[:, s],
                    in_=st["xss"][s][:, c * VBS:(c + 1) * VBS],
                    func=AF.Identity, bias=tshift[:, s, c:c + 1],
                    scale=sscale[:, s, c:c + 1])
            ztp = ps_zt.tile([128, D], f32)
            for s in range(4):
                nc.tensor.transpose(ztp[:, s * 128:(s + 1) * 128],
                                    xnc[:, s], ident)
            z = z_pool.tile([128, D], f32, tag="z")
            if c in ZMULT_ON_GP:
                # GpSimd cannot read PSUM: ACT copies, GP multiplies in place
                nc.scalar.copy(z, ztp)
                nc.gpsimd.tensor_tensor(z, z, st["p4"][:, c], OP.mult)
            else:
                nc.vector.tensor_tensor(z, ztp, st["p4"][:, c], OP.mult)
            st["z"][c] = z

            # top-16 extraction into t16[:, c, :]
            t16 = st["t16"]
            cs = st["cs"]
            l1 = sm_pool.tile([128, 32], f32, tag="l1")
            for s in range(4):
                nc.vector.max(out=l1[:, s * 8:(s + 1) * 8],
                              in_=z[:, s * 128:(s + 1) * 128])
            nc.vector.max(out=t16[:, c, 0:8], in_=l1)
            sup = sm_pool.tile([128, 32], f32, tag="sup")
            nc.vector.match_replace(sup, t16[:, c, 0:8], l1, NEG)
            nc.vector.max(out=t16[:, c, 8:16], in_=sup)
            nc.vector.tensor_tensor_scan(cs[:, c], t16[:, c], t16[:, c],
                                         -1.0, OP.add, OP.bypass)

        def emit_tail(st, sci):
            # batched tau computation over [128, 4, 16]
            t16 = st["t16"]
            cs = st["cs"]
            rz = sm_pool.tile([128, 4, 16], f32, tag="rz")
            nc.gpsimd.tensor_tensor(rz, t16, rho16p, OP.mult)
            cond = sm_pool.tile([128, 4, 17], f32, tag="cond")
            nc.gpsimd.memset(cond[:, :, 16:17], 0.0)
            nc.vector.tensor_tensor(cond[:, :, 0:16], rz, cs, OP.is_gt)
            dcn = sm_pool.tile([128, 4, 16], f32, tag="dcn")
            nc.gpsimd.tensor_tensor(dcn, cond[:, :, 1:17], cond[:, :, 0:16],
                                    OP.subtract)
            tauj = sm_pool.tile([128, 4, 16], f32, tag="tauj")
            nc.gpsimd.tensor_tensor(tauj, cs, invrhop, OP.mult)
            scr = sm_pool.tile([128, 4, 16], f32, tag="scr")
            nc.gpsimd.tensor_tensor(scr, tauj, dcn, OP.mult)
            negtau = sm_pool.tile([128, 4], f32, tag="negtau")
            nc.vector.tensor_reduce(out=negtau, in_=scr,
                                    axis=mybir.AxisListType.X, op=OP.add)
            o4 = o_pool.tile([128, 4, D], f32, name="o4", tag="o4")
            for c in range(4):
                if c in RELU_ON_DVE:
                    nc.vector.tensor_scalar(o4[:, c], st["z"][c],
                                            negtau[:, c:c + 1], 0.0,
                                            OP.add, OP.max)
                else:
                    nc.scalar.activation(o4[:, c], st["z"][c], AF.Relu,
                                         bias=negtau[:, c:c + 1], scale=1.0)
            # out-DMA on the GpSimd sw-DGE queue: its wait-for-relu must not
            # head-of-line-block the input DMAs on the Sync queue
            nc.gpsimd.dma_start(out=out_r[sci], in_=o4)

        # ---------- main loop (1-sc software pipeline skew) ----------
        def start_pend(st):
            st["z"] = {}
            st["t16"] = sm_pool.tile([128, 4, 16], f32, name="t16", tag="t16")
            st["cs"] = sm_pool.tile([128, 4, 16], f32, name="cs", tag="cs")

        pend = None
        for sc in range(N_SC):
            st = emit_load(sc)
            if pend is not None:
                start_pend(pend)
            for i in range(4):
                if pend is not None:
                    emit_chunk(pend, i)
                if i < 2:
                    emit_gemm_stats(st, i)
            emit_stats_merge(st)
            if pend is not None:
                emit_tail(pend, sc - 1)
            pend = st
        start_pend(pend)
        for i in range(4):
            emit_chunk(pend, i)
        emit_tail(pend, N_SC - 1)

    nc.compile()
    return nc


_NC_CACHE = None


def kernel(**inputs) -> np.ndarray:
    global _NC_CACHE
    if _NC_CACHE is None:
        _NC_CACHE = build_nc()
    nc = _NC_CACHE

    priors = np.ascontiguousarray(inputs["priors"], dtype=np.float32)
    feat = np.ascontiguousarray(inputs["processed_feat"], dtype=np.float32)
    W = np.ascontiguousarray(inputs["W"], dtype=np.float32)
    gamma = np.ascontiguousarray(inputs["gamma"], dtype=np.float32)
    beta = np.ascontiguousarray(inputs["beta"], dtype=np.float32)

    in_maps = []
    for i in range(N_CORES):
        sl = slice(i * ROWS, (i + 1) * ROWS)
        in_maps.append({
            "priors": priors[sl],
            "processed_feat": feat[sl],
            "W": W,
            "gamma": gamma,
            "beta": beta,
        })
    res = run_bass_kernel_spmd(nc, in_maps, core_ids=list(range(N_CORES)))
    return np.concatenate([r["out"] for r in res.results], axis=0)


if __name__ == "__main__":
    rng = np.random.default_rng(0)
    inputs = {
        "priors": rng.random((B, D), dtype=np.float32),
        "processed_feat": rng.standard_normal((B, IN), dtype=np.float32),
        "W": (rng.standard_normal((D, IN), dtype=np.float32) * 0.1),
        "gamma": np.ones(D, dtype=np.float32),
        "beta": np.zeros(D, dtype=np.float32),
    }
    out = kernel(**inputs)
    print("out", out.shape, out.dtype, float(out.sum()))
